# revision 1
# baseline (speedup 1.0000x reference)
"""DeepSeek-style 2-layer MLA transformer forward on 8 Trainium2 NeuronCores.

Sharding: sequence-parallel (data-parallel over tokens) with zigzag query-block
assignment for causal load balance.  Core c owns token blocks (c, NBLK-1-c) of
NBLK=16 blocks of 128 tokens.  Per layer there is ONE AllGather of the
rms-normed activations (needed to build K/V for all tokens); q / attention /
wo / FFN / head are computed locally for the core's 256 tokens with replicated
weights, so no all-reduce is needed anywhere.  The head is token-sharded too:
each core emits logits [256, 32000] and the host reassembles.

Activations are feature-major ([features on partitions, tokens free]) so every
matmul consumes natural-layout weights as the stationary operand.  Attention
scores are computed key-major (S^T[t, s]) so the softmax denominator is a
ones-matmul and P^T needs no transpose for the PV matmul; V is produced
token-major directly by the wkv_b matmul.  SPMD requires one program for all
cores, so the attention loop has a uniform shape (8 key blocks for the low
query block, 16 for the high one) and per-core {0, tri, 1} mask *data* does
the causal selection.  Matmuls are bf16 with fp32 PSUM accumulation; the
residual stream and softmax stats stay fp32.  Norm weights and the score
scale are folded into adjacent weight matrices on the host.
"""

import numpy as np
import ml_dtypes

import concourse.bass as bass
import concourse.mybir as mybir
import concourse.tile as tile
from concourse import bacc
from concourse.bass import IndirectOffsetOnAxis
from concourse.bass_utils import run_bass_kernel_spmd
from concourse.masks import make_identity

F32 = mybir.dt.float32
BF16 = mybir.dt.bfloat16
I32 = mybir.dt.int32
AF = mybir.ActivationFunctionType
ALU = mybir.AluOpType
P = 128
NPBF16 = ml_dtypes.bfloat16


class Cfg:
    def __init__(self, n_cores=8, S=2048, DIM=2048, H=16, KVR=512, INTER=8192,
                 VOCAB=32000, L=2):
        self.n_cores = n_cores
        self.S, self.DIM, self.H, self.KVR = S, DIM, H, KVR
        self.INTER, self.VOCAB, self.L = INTER, VOCAB, L
        self.DN, self.DR, self.DV = 128, 64, 128
        self.QKD = self.DN + self.DR
        self.SCALE = self.QKD ** -0.5
        self.NBLK = S // P
        assert self.NBLK == 2 * n_cores
        self.TLOC = 2 * P
        self.KT = DIM // P
        self.KVT = KVR // P
        self.IT = INTER // P
        self.PE_T = H // 2
        assert H % 4 == 0 and KVR % P == 0 and DIM % 512 == 0
        assert INTER % 512 == 0 and VOCAB % 256 == 0

    def blocks(self, c):
        return (c, self.NBLK - 1 - c)

    def col_of(self, kb):
        """Token-column offset of key block kb in slab (AllGather) order."""
        if kb < self.n_cores:
            return self.TLOC * kb
        return self.TLOC * (self.NBLK - 1 - kb) + P

    def pos_of(self, c):
        b0, b1 = self.blocks(c)
        return np.concatenate([np.arange(P * b0, P * b0 + P),
                               np.arange(P * b1, P * b1 + P)])

    @property
    def n_units0(self):
        return self.n_cores          # key blocks processed for low q-block

    @property
    def n_units1(self):
        return self.NBLK             # key blocks processed for high q-block


# ---------------------------------------------------------------------------
# host-side input preparation
# ---------------------------------------------------------------------------

def _deinterleave_pe(w_pe):
    """[..., 64] interleaved rope dims -> [evens(32) | odds(32)]."""
    return np.concatenate([w_pe[..., 0::2], w_pe[..., 1::2]], axis=-1)


def host_prepare(inputs, c_: Cfg):
    f = lambda a: np.ascontiguousarray(np.asarray(a, np.float32))
    bf = lambda a: np.ascontiguousarray(np.asarray(a).astype(NPBF16))

    tokens = np.asarray(inputs["tokens"]).reshape(-1)
    fc, fs = f(inputs["freqs_cos"]), f(inputs["freqs_sin"])       # [S, 32]

    perm_all = np.concatenate([c_.pos_of(r) for r in range(c_.n_cores)])
    cosT_all, sinT_all = fc[perm_all].T, fs[perm_all].T           # [32, S]
    cc_k = f(np.concatenate([cosT_all, cosT_all], 0))             # [64, S]
    ss_k = f(np.concatenate([-sinT_all, sinT_all], 0))

    tri = (np.arange(P)[:, None] <= np.arange(P)[None, :]).astype(np.float32)
    onesb = np.ones((P, P), np.float32)
    zerb = np.zeros((P, P), np.float32)

    shared = {"cc_k": cc_k, "ss_k": ss_k, "embed": f(inputs["embed"])}

    for l in range(c_.L):
        an = f(inputs["attn_norm_w"][l])[:, None]
        wq = f(inputs["wq"][l]) * an * c_.SCALE
        wq = wq.reshape(c_.DIM, c_.H, c_.QKD)
        wq_nope = wq[:, :, :c_.DN].reshape(c_.DIM, c_.H * c_.DN)
        wq_pe = _deinterleave_pe(wq[:, :, c_.DN:]).reshape(c_.DIM,
                                                          c_.H * c_.DR)
        shared[f"wq{l}"] = bf(np.concatenate([wq_nope, wq_pe], 1))

        wkva = f(inputs["wkv_a"][l]) * an
        wkva_pe = _deinterleave_pe(wkva[:, c_.KVR:])
        shared[f"wkva{l}"] = bf(np.concatenate([wkva[:, :c_.KVR], wkva_pe], 1))

        kvn = f(inputs["kv_norm_w"][l])[:, None]
        wkvb = (f(inputs["wkv_b"][l]) * kvn).reshape(c_.KVR, c_.H,
                                                     c_.DN + c_.DV)
        wb_n = wkvb[:, :, :c_.DN].reshape(c_.KVR, c_.H * c_.DN)
        wb_v = wkvb[:, :, c_.DN:].reshape(c_.KVR, c_.H * c_.DV)
        shared[f"wkvb{l}"] = bf(np.concatenate([wb_n, wb_v], 1))

        shared[f"wo{l}"] = bf(inputs["wo"][l])
        fn = f(inputs["ffn_norm_w"][l])[:, None]
        shared[f"w1{l}"] = bf(f(inputs["w1"][l]) * fn)
        shared[f"w3{l}"] = bf(f(inputs["w3"][l]) * fn)
        shared[f"w2{l}"] = bf(inputs["w2"][l])

    shared["headw"] = bf(f(inputs["head_w"]) * f(inputs["norm_w"])[:, None])

    in_maps = []
    for c in range(c_.n_cores):
        pos = c_.pos_of(c)
        cosT, sinT = fc[pos].T, fs[pos].T                         # [32, 256]
        m = dict(shared)
        m["tok"] = np.ascontiguousarray(
            tokens[pos].reshape(c_.TLOC, 1).astype(np.int32))
        m["cc_q"] = f(np.concatenate([cosT] * 4, 0))
        m["ss_q"] = f(np.concatenate([-sinT, sinT, -sinT, sinT], 0))
        # block masks: unit u covers (qb_i=0, kb=0..n_cores-1) then
        # (qb_i=1, kb=0..NBLK-1).  pattern: 1 if kb<qb, tri if kb==qb, 0 else.
        bm = []
        for qb_i, qb in enumerate(c_.blocks(c)):
            nu = c_.n_units0 if qb_i == 0 else c_.n_units1
            for kb in range(nu):
                bm.append(onesb if kb < qb else (tri if kb == qb else zerb))
        m["bmask"] = bf(np.concatenate(bm, 0))    # [(n0+n1)*128, 128]
        in_maps.append(m)
    return in_maps


# ---------------------------------------------------------------------------
# device program
# ---------------------------------------------------------------------------

def build(nc, c_: Cfg):
    S, DIM, H, KVR, INTER, VOCAB = (c_.S, c_.DIM, c_.H, c_.KVR, c_.INTER,
                                    c_.VOCAB)
    DR = c_.DR
    NU = c_.n_units0 + c_.n_units1
    d = {}
    d["tok"] = nc.dram_tensor("tok", [c_.TLOC, 1], I32, kind="ExternalInput")
    d["emb"] = nc.dram_tensor("embed", [VOCAB, DIM], F32, kind="ExternalInput")
    d["ccq"] = nc.dram_tensor("cc_q", [P, c_.TLOC], F32, kind="ExternalInput")
    d["ssq"] = nc.dram_tensor("ss_q", [P, c_.TLOC], F32, kind="ExternalInput")
    d["cck"] = nc.dram_tensor("cc_k", [DR, S], F32, kind="ExternalInput")
    d["ssk"] = nc.dram_tensor("ss_k", [DR, S], F32, kind="ExternalInput")
    d["bmask"] = nc.dram_tensor("bmask", [NU * P, P], BF16,
                                kind="ExternalInput")
    for l in range(c_.L):
        d[f"wq{l}"] = nc.dram_tensor(f"wq{l}", [DIM, H * c_.QKD], BF16,
                                     kind="ExternalInput")
        d[f"wkva{l}"] = nc.dram_tensor(f"wkva{l}", [DIM, KVR + DR], BF16,
                                       kind="ExternalInput")
        d[f"wkvb{l}"] = nc.dram_tensor(f"wkvb{l}", [KVR, H * 256], BF16,
                                       kind="ExternalInput")
        d[f"wo{l}"] = nc.dram_tensor(f"wo{l}", [H * c_.DV, DIM], BF16,
                                     kind="ExternalInput")
        d[f"w1{l}"] = nc.dram_tensor(f"w1{l}", [DIM, INTER], BF16,
                                     kind="ExternalInput")
        d[f"w2{l}"] = nc.dram_tensor(f"w2{l}", [INTER, DIM], BF16,
                                     kind="ExternalInput")
        d[f"w3{l}"] = nc.dram_tensor(f"w3{l}", [DIM, INTER], BF16,
                                     kind="ExternalInput")
    d["hw"] = nc.dram_tensor("headw", [DIM, VOCAB], BF16, kind="ExternalInput")
    d["out"] = nc.dram_tensor("logits", [c_.TLOC, VOCAB], F32,
                              kind="ExternalOutput")

    with tile.TileContext(nc) as tc:
        _emit(nc, tc, c_, d)
    nc.compile()


def _emit(nc, tc, c_: Cfg, d):
    S, DIM, H, KVR, INTER, VOCAB = (c_.S, c_.DIM, c_.H, c_.KVR, c_.INTER,
                                    c_.VOCAB)
    KT, KVT, IT, TLOC, NBLK = c_.KT, c_.KVT, c_.IT, c_.TLOC, c_.NBLK
    DR, DV = c_.DR, c_.DV
    NCOR = c_.n_cores
    NU = c_.n_units0 + c_.n_units1

    import contextlib
    stack = contextlib.ExitStack()
    cpool = stack.enter_context(tc.tile_pool(name="const", bufs=1))
    psum = stack.enter_context(tc.tile_pool(name="psum", bufs=1, space="PSUM"))
    dram = stack.enter_context(tc.tile_pool(name="dram", bufs=1, space="DRAM"))
    hpool = stack.enter_context(tc.tile_pool(name="hres", bufs=1))
    spool = stack.enter_context(tc.tile_pool(name="small", bufs=1))
    _nopex = (None, None, None)

    # ---- constants ----
    ident = cpool.tile([P, P], F32, name="ident")
    make_identity(nc, ident[:])
    ones_bf = cpool.tile([P, 1], BF16, name="ones_bf")
    nc.gpsimd.memset(ones_bf[:], 1.0)
    eps_t = cpool.tile([P, 1], F32, name="eps_t")
    nc.gpsimd.memset(eps_t[:], 1e-6)
    ccq = cpool.tile([P, TLOC], F32, name="ccq_sb")
    nc.sync.dma_start(ccq[:], d["ccq"][:, :])
    ssq = cpool.tile([P, TLOC], F32, name="ssq_sb")
    nc.sync.dma_start(ssq[:], d["ssq"][:, :])
    cck = cpool.tile([DR, S], F32, name="cck_sb")
    nc.sync.dma_start(cck[:DR, :], d["cck"][:, :])
    ssk = cpool.tile([DR, S], F32, name="ssk_sb")
    nc.sync.dma_start(ssk[:DR, :], d["ssk"][:, :])
    bm = []
    for u in range(NU):
        t = cpool.tile([P, P], BF16, name=f"bm{u}")
        nc.sync.dma_start(t[:], d["bmask"][u * P:(u + 1) * P, :])
        bm.append(t)

    # ---- residual stream ----
    hT = [hpool.tile([P, TLOC], F32, name=f"hT{i}") for i in range(KT)]

    for rep_i in range(getattr(c_, 'repeat', 1)):
        # ---- embedding gather + transpose to feature-major ----
        with tc.tile_pool(name="emb", bufs=1) as epool:
            for b in range(2):
                idx = epool.tile([P, 1], I32, name=f"idx{b}", tag="idx", bufs=2)
                nc.sync.dma_start(idx[:], d["tok"][b * P:(b + 1) * P, :])
                gth = epool.tile([P, DIM], F32, name=f"gth{b}", tag="gth", bufs=2)
                nc.gpsimd.indirect_dma_start(
                    out=gth[:], out_offset=None, in_=d["emb"][:, :],
                    in_offset=IndirectOffsetOnAxis(ap=idx[:, :1], axis=0))
                for k in range(KT):
                    tp = psum.tile([P, P], F32, name=f"tp{b}_{k}", tag="sc",
                                   bufs=2)
                    nc.tensor.transpose(tp[:], gth[:, k * P:(k + 1) * P], ident[:])
                    nc.vector.tensor_copy(hT[k][:, b * P:(b + 1) * P], tp[:])

        # ---- helpers ----
        def rms(tiles, out_tiles, pool, nm):
            """out = in * rsqrt(mean_over_partition_tiles(in^2) + 1e-6), bf16."""
            nkt = len(tiles)
            W = tiles[0].shape[-1]
            ssq_ps = psum.tile([1, W], F32, name=f"ssqp_{nm}", tag="attv", bufs=2)
            sqs = []
            for i, t in enumerate(tiles):
                sq = pool.tile([P, W], BF16, name=f"sq_{nm}_{i}", tag=f"sq_{nm}",
                               bufs=3)
                nc.vector.tensor_tensor(out=sq[:], in0=t[:], in1=t[:], op=ALU.mult)
                sqs.append(sq)
            for i, sq in enumerate(sqs):
                nc.tensor.matmul(ssq_ps[:1, :], ones_bf[:, :1], sq[:],
                                 start=(i == 0), stop=(i == nkt - 1))
            srt = spool.tile([1, W], F32, name=f"srt_{nm}", tag="srt", bufs=2)
            nc.scalar.activation(srt[:1, :], ssq_ps[:1, :], AF.Sqrt,
                                 bias=eps_t[:1, :1], scale=1.0 / (nkt * P))
            rcp = spool.tile([1, W], F32, name=f"rcp_{nm}", tag="rcp", bufs=2)
            nc.vector.reciprocal(rcp[:1, :], srt[:1, :])
            scb = pool.tile([P, W], F32, name=f"scb_{nm}", tag=f"scb_{nm}", bufs=1)
            nc.gpsimd.partition_broadcast(scb[:], rcp[:1, :])
            for i, t in enumerate(tiles):
                nc.vector.tensor_tensor(out=out_tiles[i][:], in0=t[:], in1=scb[:],
                                        op=ALU.mult)

        def rope(ps_ap, rows, cc, ss, outs, pool, nm):
            """ps_ap: [rows, W] fp32 (psum) with [e(32)|o(32)] row groups.
            outs: list of (bf16 out AP [64, W], row0 in ps)."""
            W = ps_ap.shape[-1]
            t1 = pool.tile([P, W], F32, name=f"rt1_{nm}", tag="rt1", bufs=2)
            t2 = pool.tile([P, W], F32, name=f"rt2_{nm}", tag="rt2", bufs=2)
            nc.vector.tensor_tensor(out=t1[:rows, :], in0=ps_ap[:rows, :],
                                    in1=cc[:rows, :], op=ALU.mult)
            for g in range(rows // 32):
                sg = g ^ 1
                nc.vector.tensor_tensor(
                    out=t2[g * 32:(g + 1) * 32, :],
                    in0=ps_ap[sg * 32:(sg + 1) * 32, :],
                    in1=ss[g * 32:(g + 1) * 32, :], op=ALU.mult)
            for out_ap, r0 in outs:
                nc.vector.tensor_tensor(out=out_ap, in0=t1[r0:r0 + 64, :],
                                        in1=t2[r0:r0 + 64, :], op=ALU.add)

        # =======================================================================
        for l in range(c_.L):
            # Pool scoping (LIFO): pA holds q/kpe through attention; pD holds
            # kv_norm through kvb; pB (xT) closes after q; pC (kv_a temps)
            # closes after kv norm; pE (v/attention/wo) closes after wo;
            # pF (FFN) is last.
            pA_cm = tc.tile_pool(name=f"qkv{l}", bufs=1)
            pA = pA_cm.__enter__()
            pD_cm = tc.tile_pool(name=f"kvnorm{l}", bufs=1)
            pD = pD_cm.__enter__()
            pB_cm = tc.tile_pool(name=f"x{l}", bufs=1)
            pB = pB_cm.__enter__()
            if True:
                # ---------- rms 1 -> xT ----------
                xTt = pB.tile([P, KT, TLOC], BF16, name=f"xT{l}", tag="xT")
                xT = [xTt[:, i, :] for i in range(KT)]
                rms(hT, xT, pB, f"a{l}")

                # ---------- AllGather xT ----------
                ag_in = dram.tile([DIM, TLOC], BF16, name=f"agin{l}", tag="agin",
                                  bufs=2)
                ag_out = dram.tile([NCOR * DIM, TLOC], BF16, name=f"agout{l}",
                                   tag="agout", bufs=2,
                                   addr_space="Shared"
                                   if (NCOR > 4 and not getattr(c_, "mock_ag",
                                                                False))
                                   else "Local")
                nc.sync.dma_start(
                    ag_in.rearrange("(kt p) c -> p kt c", p=P), xTt[:, :, :])
                if getattr(c_, "mock_ag", False):
                    # timing-model variant: stand in for the AllGather with
                    # local DMA copies (data is wrong for r != my core)
                    for r in range(NCOR):
                        nc.sync.dma_start(ag_out[r * DIM:(r + 1) * DIM, :],
                                          ag_in[:, :])
                else:
                    nc.gpsimd.collective_compute(
                        "AllGather", ALU.bypass,
                        replica_groups=[list(range(NCOR))],
                        ins=[ag_in.opt()], outs=[ag_out.opt()])

                # ---------- q projection (local tokens; overlaps AllGather) ----
                qn = [pA.tile([P, TLOC], BF16, name=f"qn{l}_{h}", tag=f"qn{h}")
                      for h in range(H)]
                qpe = [pA.tile([64, TLOC], BF16, name=f"qpe{l}_{h}", tag=f"qpe{h}")
                       for h in range(H)]
                MQ = H + c_.PE_T
                wq_r = d[f"wq{l}"].rearrange("(kt p) c -> p kt c", p=P)
                for mg in range((MQ + 3) // 4):
                    mw = min(512, MQ * P - mg * 512)
                    wqs = pB.tile([P, KT, 512], BF16, name=f"wq{l}_{mg}",
                                  tag="wq", bufs=2)
                    nc.sync.dma_start(wqs[:, :, :mw],
                                      wq_r[:, :, mg * 512:mg * 512 + mw])
                    for mi in range(4):
                        m = mg * 4 + mi
                        if m >= MQ:
                            break
                        ps = psum.tile([P, TLOC], F32, name=f"qps{l}_{m}",
                                       tag="mm", bufs=3)
                        for k in range(KT):
                            nc.tensor.matmul(ps[:],
                                             wqs[:, k, mi * P:(mi + 1) * P],
                                             xT[k][:], start=(k == 0),
                                             stop=(k == KT - 1))
                        if m < H:
                            nc.any.tensor_copy(qn[m][:], ps[:])
                        else:
                            p = m - H
                            rope(ps[:], P, ccq, ssq,
                                 [(qpe[2 * p][:64, :], 0),
                                  (qpe[2 * p + 1][:64, :], 64)], pB, f"q{l}_{p}")
            pB_cm.__exit__(*_nopex)

            pC_cm = tc.tile_pool(name=f"kva{l}", bufs=1)
            pC = pC_cm.__enter__()
            if True:
                # ---------- kv_a over gathered tokens ----------
                kpe = pA.tile([64, S], BF16, name=f"kpe{l}", tag="kpe")
                kvf = [pC.tile([P, S], BF16, name=f"kvf{l}_{m}", tag=f"kvf{m}")
                       for m in range(KVT)]
                kp = pC
                wkva_sb = kp.tile([P, KT, KVR + DR], BF16, name=f"wkva{l}",
                                  tag="wkva", bufs=1)
                nc.sync.dma_start(
                    wkva_sb[:, :, :],
                    d[f"wkva{l}"].rearrange("(kt p) c -> p kt c", p=P))
                ago_r = ag_out.rearrange("(r kt p) c -> r p kt c", r=NCOR, p=P)
                for r in range(NCOR):
                    xg = kp.tile([P, KT, TLOC], BF16, name=f"xg{l}_{r}",
                                 tag="xg", bufs=2)
                    nc.sync.dma_start(xg[:, :, :], ago_r[r])
                    for m in range(KVT):
                        ps = psum.tile([P, TLOC], F32, name=f"kvps{l}_{r}_{m}",
                                       tag="mm", bufs=3)
                        for k in range(KT):
                            nc.tensor.matmul(
                                ps[:], wkva_sb[:, k, m * P:(m + 1) * P],
                                xg[:, k, :], start=(k == 0), stop=(k == KT - 1))
                        nc.scalar.copy(kvf[m][:, r * TLOC:(r + 1) * TLOC],
                                       ps[:])
                    # rope part (M = 64)
                    ps = psum.tile([P, TLOC], F32, name=f"kpps{l}_{r}",
                                   tag="mm", bufs=3)
                    for k in range(KT):
                        nc.tensor.matmul(ps[:64, :],
                                         wkva_sb[:, k, KVR:KVR + DR],
                                         xg[:, k, :], start=(k == 0),
                                         stop=(k == KT - 1))
                    rope(ps[:], 64, cck[:, r * TLOC:(r + 1) * TLOC],
                         ssk[:, r * TLOC:(r + 1) * TLOC],
                         [(kpe[:64, r * TLOC:(r + 1) * TLOC], 0)],
                         kp, f"k{l}_{r}")

                # ---------- kv rms norm ----------
                kvn = [pD.tile([P, S], BF16, name=f"kvn{l}_{m}", tag=f"kvn{m}")
                       for m in range(KVT)]
                sqs = [pC.tile([P, S], BF16, name=f"kvsq{l}_{m}",
                               tag=f"kvsq{m}") for m in range(KVT)]
                for m in range(KVT):
                    nc.vector.tensor_tensor(out=sqs[m][:], in0=kvf[m][:],
                                            in1=kvf[m][:], op=ALU.mult)
                sckv = pC.tile([1, S], F32, name=f"sckv{l}", tag="sckv")
                for r in range(NCOR):
                    ssq_ps = psum.tile([1, TLOC], F32, name=f"kvssq{l}_{r}",
                                       tag="attv", bufs=2)
                    for m in range(KVT):
                        nc.tensor.matmul(
                            ssq_ps[:1, :], ones_bf[:, :1],
                            sqs[m][:, r * TLOC:(r + 1) * TLOC],
                            start=(m == 0), stop=(m == KVT - 1))
                    srt = spool.tile([1, TLOC], F32, name=f"kvsrt{l}_{r}",
                                     tag="srt", bufs=2)
                    nc.scalar.activation(srt[:1, :], ssq_ps[:1, :], AF.Sqrt,
                                         bias=eps_t[:1, :1], scale=1.0 / KVR)
                    nc.vector.reciprocal(sckv[:1, r * TLOC:(r + 1) * TLOC],
                                         srt[:1, :])
                scbkv = pC.tile([P, S], F32, name=f"scbkv{l}", tag="scbkv")
                nc.gpsimd.partition_broadcast(scbkv[:], sckv[:1, :])
                for m in range(KVT):
                    nc.vector.tensor_tensor(out=kvn[m][:], in0=kvf[m][:],
                                            in1=scbkv[:], op=ALU.mult)
            pC_cm.__exit__(*_nopex)

            pV_cm = tc.tile_pool(name=f"vres{l}", bufs=1)
            pV = pV_cm.__enter__()
            pE_cm = tc.tile_pool(name=f"vat{l}", bufs=1)
            pE = pE_cm.__enter__()
            if True:
                # ---------- wkv_b: k_nope (-> DRAM) and V (token-major) -------
                knd = dram.tile([H * P, S], BF16, name=f"knd{l}", tag="knd",
                                bufs=2)
                vsb = [pV.tile([P, H * DV], BF16, name=f"vsb{l}_{t}",
                               tag=f"vsb{t}") for t in range(NBLK)]
                bp = pE
                wkvb_r = d[f"wkvb{l}"].rearrange("(kt p) c -> p kt c", p=P)
                for mg in range(H // 4):
                    wn = bp.tile([P, KVT, 512], BF16, name=f"wn{l}_{mg}",
                                 tag="wvb", bufs=2)
                    nc.sync.dma_start(wn[:, :, :],
                                      wkvb_r[:, :, mg * 512:mg * 512 + 512])
                    for mi in range(4):
                        h = mg * 4 + mi
                        kst = bp.tile([P, S], BF16, name=f"kst{l}_{h}",
                                      tag="kst", bufs=2)
                        for r in range(NCOR):
                            ps = psum.tile([P, TLOC], F32,
                                           name=f"knps{l}_{h}_{r}",
                                           tag="mm", bufs=3)
                            for k in range(KVT):
                                nc.tensor.matmul(
                                    ps[:], wn[:, k, mi * P:(mi + 1) * P],
                                    kvn[k][:, r * TLOC:(r + 1) * TLOC],
                                    start=(k == 0), stop=(k == KVT - 1))
                            nc.any.tensor_copy(
                                kst[:, r * TLOC:(r + 1) * TLOC], ps[:])
                        nc.sync.dma_start(knd[h * P:(h + 1) * P, :], kst[:])
                for vc in range(H * DV // 512):
                    wv = bp.tile([P, KVT, 512], BF16, name=f"wv{l}_{vc}",
                                 tag="wvb", bufs=2)
                    nc.sync.dma_start(
                        wv[:, :, :],
                        wkvb_r[:, :, H * c_.DN + vc * 512:
                               H * c_.DN + vc * 512 + 512])
                    for t in range(NBLK):
                        ps = psum.tile([P, 512], F32, name=f"vps{l}_{vc}_{t}",
                                       tag="mm", bufs=3)
                        for k in range(KVT):
                            nc.tensor.matmul(
                                ps[:], kvn[k][:, t * P:(t + 1) * P],
                                wv[:, k, :], start=(k == 0), stop=(k == KVT - 1))
                        nc.any.tensor_copy(vsb[t][:, vc * 512:vc * 512 + 512],
                                           ps[:])

                # ---------- attention ----------
                aat = [pV.tile([P, TLOC], BF16, name=f"aat{l}_{i}", tag=f"aat{i}")
                       for i in range(KT)]
                ap = pE
                if True:
                    for qb_i in range(2):
                        nkb = c_.n_units0 if qb_i == 0 else c_.n_units1
                        u0 = 0 if qb_i == 0 else c_.n_units0
                        scol = qb_i * P
                        for h in range(H):
                            knr = knd[h * P:(h + 1) * P, :].rearrange(
                                "p (u c) -> p u c", c=P)
                            kn = ap.tile([P, nkb, P], BF16,
                                         name=f"kn{l}_{qb_i}_{h}",
                                         tag=f"kn{qb_i}", bufs=2)
                            if qb_i == 1:
                                nc.sync.dma_start(kn[:, :, :], knr[:, :, :])
                            else:
                                # low q-block needs only the first half-block of
                                # each slab pair: columns [2*u*P, 2*u*P + P)
                                nc.sync.dma_start(
                                    kn[:, :, :],
                                    knd[h * P:(h + 1) * P, :].rearrange(
                                        "p (u c) -> p u c", c=2 * P)[:, :, :P])
                            att_ps = psum.tile([P, P], F32,
                                               name=f"atp{l}_{qb_i}_{h}",
                                               tag="attv", bufs=2)
                            den_ps = psum.tile([1, P], F32,
                                               name=f"dnp{l}_{qb_i}_{h}",
                                               tag="den", bufs=1)
                            for j in range(nkb):
                                kcol = c_.col_of(j)
                                sc_ps = psum.tile([P, P], F32,
                                                  name=f"scp{l}_{qb_i}_{h}_{j}",
                                                  tag="sc", bufs=2)
                                kj = kcol // P if qb_i == 1 else j
                                nc.tensor.matmul(sc_ps[:], kn[:, kj, :],
                                                 qn[h][:, scol:scol + P],
                                                 start=True, stop=False)
                                nc.tensor.matmul(sc_ps[:],
                                                 kpe[:64, kcol:kcol + P],
                                                 qpe[h][:64, scol:scol + P],
                                                 start=False, stop=True)
                                e = ap.tile([P, P], BF16,
                                            name=f"e{l}_{qb_i}_{h}_{j}",
                                            tag="e", bufs=6)
                                nc.scalar.activation(e[:], sc_ps[:], AF.Exp)
                                if qb_i == 0 or j >= c_.n_units0:
                                    nc.vector.tensor_tensor(out=e[:], in0=e[:],
                                                            in1=bm[u0 + j][:],
                                                            op=ALU.mult)
                                tt = kcol // P
                                nc.tensor.matmul(att_ps[:],
                                                 vsb[tt][:, h * DV:(h + 1) * DV],
                                                 e[:], start=(j == 0),
                                                 stop=(j == nkb - 1))
                                nc.tensor.matmul(den_ps[:1, :], ones_bf[:, :1],
                                                 e[:], start=(j == 0),
                                                 stop=(j == nkb - 1))
                            rcp = spool.tile([1, P], F32,
                                             name=f"arc{l}_{qb_i}_{h}", tag="rcp",
                                             bufs=2)
                            nc.vector.reciprocal(rcp[:1, :], den_ps[:1, :])
                            rb = ap.tile([P, P], F32, name=f"rb{l}_{qb_i}_{h}",
                                         tag="rb", bufs=2)
                            nc.gpsimd.partition_broadcast(rb[:], rcp[:1, :])
                            nc.vector.tensor_tensor(
                                out=aat[h][:, scol:scol + P], in0=att_ps[:],
                                in1=rb[:], op=ALU.mult)

                # ---------- wo + residual ----------
                op = pE
                if True:
                    wo_r = d[f"wo{l}"].rearrange("(kt p) c -> p kt c", p=P)
                    KH = min(8, KT)
                    for mg in range(KT // 4):
                        wos = op.tile([P, KT, 512], BF16, name=f"wo{l}_{mg}",
                                      tag="wo", bufs=1)
                        for kh in range(KT // KH):
                            nc.sync.dma_start(
                                wos[:, kh * KH:(kh + 1) * KH, :],
                                wo_r[:, kh * KH:(kh + 1) * KH,
                                     mg * 512:mg * 512 + 512])
                        for mi in range(4):
                            m = mg * 4 + mi
                            ps = psum.tile([P, TLOC], F32, name=f"ops{l}_{m}",
                                           tag="mm", bufs=3)
                            for k in range(KT):
                                nc.tensor.matmul(
                                    ps[:], wos[:, k, mi * P:(mi + 1) * P],
                                    aat[k][:], start=(k == 0), stop=(k == KT - 1))
                            nc.vector.tensor_tensor(out=hT[m][:], in0=hT[m][:],
                                                    in1=ps[:], op=ALU.add)

            pE_cm.__exit__(*_nopex)
            pV_cm.__exit__(*_nopex)
            pD_cm.__exit__(*_nopex)
            pA_cm.__exit__(*_nopex)

            pF_cm = tc.tile_pool(name=f"ffn{l}", bufs=1)
            pF = pF_cm.__enter__()
            if True:
                # ---------- FFN ----------
                fp = pF
                x2T = [pF.tile([P, TLOC], BF16, name=f"x2T{l}_{i}", tag=f"xT{i}_f")
                       for i in range(KT)]
                rms(hT, x2T, pF, f"f{l}")
                gat = [pF.tile([P, TLOC], BF16, name=f"gat{l}_{m}", tag=f"gat{m}")
                       for m in range(IT)]
                w1_r = d[f"w1{l}"].rearrange("(kt p) c -> p kt c", p=P)
                w3_r = d[f"w3{l}"].rearrange("(kt p) c -> p kt c", p=P)
                KH = min(8, KT)
                for mg in range(IT // 4):
                    w1s = fp.tile([P, KT, 512], BF16, name=f"w1{l}_{mg}",
                                  tag="w1", bufs=2)
                    w3s = fp.tile([P, KT, 512], BF16, name=f"w3{l}_{mg}",
                                  tag="w3", bufs=2)
                    for kh in range(KT // KH):
                        ksl = slice(kh * KH, (kh + 1) * KH)
                        nc.sync.dma_start(w1s[:, ksl, :],
                                          w1_r[:, ksl, mg * 512:mg * 512 + 512])
                        nc.sync.dma_start(w3s[:, ksl, :],
                                          w3_r[:, ksl, mg * 512:mg * 512 + 512])
                    for mi in range(4):
                        m = mg * 4 + mi
                        ups = psum.tile([P, TLOC], F32, name=f"ups{l}_{m}",
                                        tag="mm", bufs=3)
                        for k in range(KT):
                            nc.tensor.matmul(
                                ups[:], w1s[:, k, mi * P:(mi + 1) * P],
                                x2T[k][:], start=(k == 0), stop=(k == KT - 1))
                        sg = fp.tile([P, TLOC], BF16, name=f"sg{l}_{m}",
                                     tag="sg", bufs=4)
                        nc.scalar.activation(sg[:], ups[:], AF.Sigmoid)
                        su = fp.tile([P, TLOC], BF16, name=f"su{l}_{m}",
                                     tag="su", bufs=4)
                        nc.vector.tensor_tensor(out=su[:], in0=ups[:], in1=sg[:],
                                                op=ALU.mult)
                        gps = psum.tile([P, TLOC], F32, name=f"gps{l}_{m}",
                                        tag="mm", bufs=3)
                        for k in range(KT):
                            nc.tensor.matmul(
                                gps[:], w3s[:, k, mi * P:(mi + 1) * P],
                                x2T[k][:], start=(k == 0), stop=(k == KT - 1))
                        nc.vector.tensor_tensor(out=gat[m][:], in0=gps[:],
                                                in1=su[:], op=ALU.mult)
                # w2: k-outer accumulation, m-groups of 4 (psum tags borrowed
                # from the idle attention tags to stay within 8 banks)
                w2_r = d[f"w2{l}"].rearrange("(kt p) c -> p kt c", p=P)
                KG = max(1, IT // KH)
                for mg in range(KT // 4):
                    tags = [("mm", 3), ("mm", 3), ("sc", 2), ("attv", 2)]
                    pss = [psum.tile([P, TLOC], F32, name=f"yps{l}_{mg}_{mi}",
                                     tag=tags[mi][0], bufs=tags[mi][1])
                           for mi in range(4)]
                    for kg in range(KG):
                        w2t = fp.tile([P, KH, 512], BF16, name=f"w2{l}_{mg}_{kg}",
                                      tag="w2", bufs=3)
                        nc.sync.dma_start(
                            w2t[:, :, :],
                            w2_r[:, kg * KH:(kg + 1) * KH,
                                 mg * 512:mg * 512 + 512])
                        for ki in range(KH):
                            k = kg * KH + ki
                            for mi in range(4):
                                nc.tensor.matmul(
                                    pss[mi][:], w2t[:, ki, mi * P:(mi + 1) * P],
                                    gat[k][:], start=(k == 0),
                                    stop=(k == IT - 1))
                    for mi in range(4):
                        m = mg * 4 + mi
                        nc.vector.tensor_tensor(out=hT[m][:], in0=hT[m][:],
                                                in1=pss[mi][:], op=ALU.add)
            pF_cm.__exit__(*_nopex)

        # ---------- final norm + vocab head (token-major output) ----------
        with tc.tile_pool(name="head", bufs=1) as hp:
            xfT = [hp.tile([P, TLOC], BF16, name=f"xfT{i}", tag=f"xfT{i}")
                   for i in range(KT)]
            rms(hT, xfT, hp, "h")
            NV = VOCAB // 512 if VOCAB % 512 == 0 else VOCAB // 512 + 1
            hw_r = d["hw"].rearrange("(kt p) v -> p kt v", p=P)
            for vc in range(NV):
                vw = min(512, VOCAB - vc * 512)
                hws = hp.tile([P, KT, 512], BF16, name=f"hw_{vc}", tag="hw",
                              bufs=3)
                KH = min(8, KT)
                for kh in range(KT // KH):
                    ksl = slice(kh * KH, (kh + 1) * KH)
                    nc.sync.dma_start(hws[:, ksl, :vw],
                                      hw_r[:, ksl, vc * 512:vc * 512 + vw])
                for st in range(2):
                    ps = psum.tile([P, 512], F32, name=f"lps_{vc}_{st}", tag="mm",
                                   bufs=3)
                    for k in range(KT):
                        nc.tensor.matmul(ps[:, :vw],
                                         xfT[k][:, st * P:(st + 1) * P],
                                         hws[:, k, :vw],
                                         start=(k == 0), stop=(k == KT - 1))
                    lg = hp.tile([P, 512], F32, name=f"lg_{vc}_{st}", tag="lg",
                                 bufs=4)
                    nc.any.tensor_copy(lg[:, :vw], ps[:, :vw])
                    nc.sync.dma_start(
                        d["out"][st * P:(st + 1) * P, vc * 512:vc * 512 + vw],
                        lg[:, :vw])

    stack.close()


# ---------------------------------------------------------------------------
# entry point
# ---------------------------------------------------------------------------

_CACHE = {}


def _get_nc(c_: Cfg):
    key = tuple(sorted(c_.__dict__.items()))
    if key not in _CACHE:
        nc = bacc.Bacc("TRN2", target_bir_lowering=False, debug=False,
                       num_devices=c_.n_cores)
        build(nc, c_)
        _CACHE[key] = nc
    return _CACHE[key]


def kernel(**inputs):
    c_ = Cfg()
    nc = _get_nc(c_)
    in_maps = host_prepare(inputs, c_)
    res = run_bass_kernel_spmd(nc, in_maps,
                               core_ids=list(range(c_.n_cores)))
    out = np.zeros((1, c_.S, c_.VOCAB), np.float32)
    for c in range(c_.n_cores):
        out[0, c_.pos_of(c), :] = res.results[c]["logits"]
    return out



# revision 9
# speedup vs baseline: 1.9796x; 1.9796x over previous
"""DeepSeek-style 2-layer MLA transformer forward on 8 Trainium2 NeuronCores.

Sharding: sequence-parallel (data-parallel over tokens) with zigzag query-block
assignment for causal load balance.  Core c owns token blocks (c, NBLK-1-c) of
NBLK=16 blocks of 128 tokens.  Per layer there is ONE AllGather of the
rms-normed activations (needed to build K/V for all tokens); q / attention /
wo / FFN / head are computed locally for the core's 256 tokens with replicated
weights, so no all-reduce is needed anywhere.  The head is token-sharded too:
each core emits logits [256, 32000] and the host reassembles.

Activations are feature-major ([features on partitions, tokens free]) so every
matmul consumes natural-layout weights as the stationary operand.  Attention
scores are computed key-major (S^T[t, s]) so the softmax denominator is a
ones-matmul and P^T needs no transpose for the PV matmul; V is produced
token-major directly by the wkv_b matmul.  SPMD requires one program for all
cores, so the attention loop has a uniform shape (8 key blocks for the low
query block, 16 for the high one) and per-core {0, tri, 1} mask *data* does
the causal selection.  Matmuls are bf16 with fp32 PSUM accumulation; the
residual stream and softmax stats stay fp32.  Norm weights and the score
scale are folded into adjacent weight matrices on the host.
"""

import numpy as np
import ml_dtypes

import concourse.bass as bass
import concourse.mybir as mybir
import concourse.tile as tile
from concourse import bacc
from concourse.bass import IndirectOffsetOnAxis
from concourse.bass_utils import run_bass_kernel_spmd
from concourse.masks import make_identity

F32 = mybir.dt.float32
BF16 = mybir.dt.bfloat16
I32 = mybir.dt.int32
AF = mybir.ActivationFunctionType
ALU = mybir.AluOpType
P = 128
NPBF16 = ml_dtypes.bfloat16


class Cfg:
    def __init__(self, n_cores=8, S=2048, DIM=2048, H=16, KVR=512, INTER=8192,
                 VOCAB=32000, L=2):
        self.n_cores = n_cores
        self.S, self.DIM, self.H, self.KVR = S, DIM, H, KVR
        self.INTER, self.VOCAB, self.L = INTER, VOCAB, L
        self.DN, self.DR, self.DV = 128, 64, 128
        self.QKD = self.DN + self.DR
        self.SCALE = self.QKD ** -0.5
        self.NBLK = S // P
        assert self.NBLK == 2 * n_cores
        self.TLOC = 2 * P
        self.KT = DIM // P
        self.KVT = KVR // P
        self.IT = INTER // P
        self.PE_T = H // 2
        assert H % 4 == 0 and KVR % P == 0 and DIM % 512 == 0
        assert INTER % 512 == 0 and VOCAB % 256 == 0

    def blocks(self, c):
        return (c, self.NBLK - 1 - c)

    def col_of(self, kb):
        """Token-column offset of key block kb in slab (AllGather) order."""
        if kb < self.n_cores:
            return self.TLOC * kb
        return self.TLOC * (self.NBLK - 1 - kb) + P

    def pos_of(self, c):
        b0, b1 = self.blocks(c)
        return np.concatenate([np.arange(P * b0, P * b0 + P),
                               np.arange(P * b1, P * b1 + P)])

    @property
    def n_units0(self):
        return self.n_cores          # key blocks processed for low q-block

    @property
    def n_units1(self):
        return self.NBLK             # key blocks processed for high q-block


# ---------------------------------------------------------------------------
# host-side input preparation
# ---------------------------------------------------------------------------

def _deinterleave_pe(w_pe):
    """[..., 64] interleaved rope dims -> [evens(32) | odds(32)]."""
    return np.concatenate([w_pe[..., 0::2], w_pe[..., 1::2]], axis=-1)


def host_prepare(inputs, c_: Cfg):
    f = lambda a: np.ascontiguousarray(np.asarray(a, np.float32))
    bf = lambda a: np.ascontiguousarray(np.asarray(a).astype(NPBF16))

    tokens = np.asarray(inputs["tokens"]).reshape(-1)
    fc, fs = f(inputs["freqs_cos"]), f(inputs["freqs_sin"])       # [S, 32]

    tri = (np.arange(P)[:, None] <= np.arange(P)[None, :]).astype(np.float32)
    onesb = np.ones((P, P), np.float32)
    zerb = np.zeros((P, P), np.float32)

    def blkmask(kb, qb):
        return onesb if kb < qb else (tri if kb == qb else zerb)

    shared = {"embed": f(inputs["embed"])}

    for l in range(c_.L):
        an = f(inputs["attn_norm_w"][l])[:, None]
        wq = f(inputs["wq"][l]) * an * c_.SCALE
        wq = wq.reshape(c_.DIM, c_.H, c_.QKD)
        wq_nope = wq[:, :, :c_.DN].reshape(c_.DIM, c_.H * c_.DN)
        wq_pe = _deinterleave_pe(wq[:, :, c_.DN:]).reshape(c_.DIM,
                                                          c_.H * c_.DR)
        shared[f"wq{l}"] = bf(np.concatenate([wq_nope, wq_pe], 1))

        wkva = f(inputs["wkv_a"][l]) * an
        wkva_pe = _deinterleave_pe(wkva[:, c_.KVR:])
        shared[f"wkva{l}"] = bf(np.concatenate([wkva[:, :c_.KVR], wkva_pe], 1))

        kvn = f(inputs["kv_norm_w"][l])[:, None]
        wkvb = (f(inputs["wkv_b"][l]) * kvn).reshape(c_.KVR, c_.H,
                                                     c_.DN + c_.DV)
        wb_n = wkvb[:, :, :c_.DN].reshape(c_.KVR, c_.H * c_.DN)
        wb_v = wkvb[:, :, c_.DN:].reshape(c_.KVR, c_.H * c_.DV)
        shared[f"wkvb{l}"] = bf(np.concatenate([wb_n, wb_v], 1))

        shared[f"wo{l}"] = bf(inputs["wo"][l])
        fn = f(inputs["ffn_norm_w"][l])[:, None]
        shared[f"w1{l}"] = bf(f(inputs["w1"][l]) * fn)
        shared[f"w3{l}"] = bf(f(inputs["w3"][l]) * fn)
        shared[f"w2{l}"] = bf(inputs["w2"][l])

    shared["headw"] = bf(f(inputs["head_w"]) * f(inputs["norm_w"])[:, None])

    in_maps = []
    for c in range(c_.n_cores):
        pos = c_.pos_of(c)
        cosT, sinT = fc[pos].T, fs[pos].T                         # [32, 256]
        m = dict(shared)
        m["tok"] = np.ascontiguousarray(
            tokens[pos].reshape(c_.TLOC, 1).astype(np.int32))
        m["cc_q"] = f(np.concatenate([cosT] * 4, 0))
        m["ss_q"] = f(np.concatenate([-sinT, sinT, -sinT, sinT], 0))
        m["cc_kl"] = f(np.concatenate([cosT, cosT], 0))           # [64, 256]
        m["ss_kl"] = f(np.concatenate([-sinT, sinT], 0))
        # block masks per key block kb: [128 keys, 256 q] where q cols
        # 0:128 are the low q-block (qb0=c) and 128:256 the high (qb1).
        # kb<8: [mask(kb,qb0) | ones]; kb>=8: [zeros | mask(kb,qb1)]
        # (kb>=8 never contributes to qb0; the zero mask keeps the single
        # [*,256] psum accumulation over all 16 kb correct).
        qb0, qb1 = c_.blocks(c)
        bm = []
        for kb in range(c_.NBLK):
            if kb < c_.n_cores:
                bm.append(np.concatenate([blkmask(kb, qb0), onesb], 1))
            else:
                bm.append(np.concatenate([zerb, blkmask(kb, qb1)], 1))
        m["bmask"] = bf(np.concatenate(bm, 0))    # [NBLK*128, 256]
        in_maps.append(m)
    return in_maps


# ---------------------------------------------------------------------------
# device program
# ---------------------------------------------------------------------------

def build(nc, c_: Cfg):
    S, DIM, H, KVR, INTER, VOCAB = (c_.S, c_.DIM, c_.H, c_.KVR, c_.INTER,
                                    c_.VOCAB)
    DR = c_.DR
    d = {}
    d["tok"] = nc.dram_tensor("tok", [c_.TLOC, 1], I32, kind="ExternalInput")
    d["emb"] = nc.dram_tensor("embed", [VOCAB, DIM], F32, kind="ExternalInput")
    d["ccq"] = nc.dram_tensor("cc_q", [P, c_.TLOC], F32, kind="ExternalInput")
    d["ssq"] = nc.dram_tensor("ss_q", [P, c_.TLOC], F32, kind="ExternalInput")
    d["cckl"] = nc.dram_tensor("cc_kl", [DR, c_.TLOC], F32,
                               kind="ExternalInput")
    d["sskl"] = nc.dram_tensor("ss_kl", [DR, c_.TLOC], F32,
                               kind="ExternalInput")
    d["bmask"] = nc.dram_tensor("bmask", [c_.NBLK * P, 2 * P], BF16,
                                kind="ExternalInput")
    for l in range(c_.L):
        d[f"wq{l}"] = nc.dram_tensor(f"wq{l}", [DIM, H * c_.QKD], BF16,
                                     kind="ExternalInput")
        d[f"wkva{l}"] = nc.dram_tensor(f"wkva{l}", [DIM, KVR + DR], BF16,
                                       kind="ExternalInput")
        d[f"wkvb{l}"] = nc.dram_tensor(f"wkvb{l}", [KVR, H * 256], BF16,
                                       kind="ExternalInput")
        d[f"wo{l}"] = nc.dram_tensor(f"wo{l}", [H * c_.DV, DIM], BF16,
                                     kind="ExternalInput")
        d[f"w1{l}"] = nc.dram_tensor(f"w1{l}", [DIM, INTER], BF16,
                                     kind="ExternalInput")
        d[f"w2{l}"] = nc.dram_tensor(f"w2{l}", [INTER, DIM], BF16,
                                     kind="ExternalInput")
        d[f"w3{l}"] = nc.dram_tensor(f"w3{l}", [DIM, INTER], BF16,
                                     kind="ExternalInput")
    d["hw"] = nc.dram_tensor("headw", [DIM, VOCAB], BF16, kind="ExternalInput")
    d["out"] = nc.dram_tensor("logits", [c_.TLOC, VOCAB], F32,
                              kind="ExternalOutput")

    with tile.TileContext(nc) as tc:
        _emit(nc, tc, c_, d)
    nc.compile()


def _emit(nc, tc, c_: Cfg, d):
    S, DIM, H, KVR, INTER, VOCAB = (c_.S, c_.DIM, c_.H, c_.KVR, c_.INTER,
                                    c_.VOCAB)
    KT, KVT, IT, TLOC, NBLK = c_.KT, c_.KVT, c_.IT, c_.TLOC, c_.NBLK
    DR, DV = c_.DR, c_.DV
    NCOR = c_.n_cores
    KVRD = KVR + DR

    import contextlib
    stack = contextlib.ExitStack()
    cpool = stack.enter_context(tc.tile_pool(name="const", bufs=1))
    psum = stack.enter_context(tc.tile_pool(name="psum", bufs=1, space="PSUM"))
    dram = stack.enter_context(tc.tile_pool(name="dram", bufs=1, space="DRAM"))
    hpool = stack.enter_context(tc.tile_pool(name="hres", bufs=1))
    spool = stack.enter_context(tc.tile_pool(name="small", bufs=1))
    _nopex = (None, None, None)

    # ---- constants ----
    ident = cpool.tile([P, P], F32, name="ident")
    make_identity(nc, ident[:])
    ones_bf = cpool.tile([P, 1], BF16, name="ones_bf")
    nc.gpsimd.memset(ones_bf[:], 1.0)
    eps_t = cpool.tile([P, 1], F32, name="eps_t")
    nc.gpsimd.memset(eps_t[:], 1e-6)
    ccq = cpool.tile([P, TLOC], F32, name="ccq_sb")
    nc.sync.dma_start(ccq[:], d["ccq"][:, :])
    ssq = cpool.tile([P, TLOC], F32, name="ssq_sb")
    nc.sync.dma_start(ssq[:], d["ssq"][:, :])
    cckl = cpool.tile([DR, TLOC], F32, name="cckl_sb")
    nc.sync.dma_start(cckl[:DR, :], d["cckl"][:, :])
    sskl = cpool.tile([DR, TLOC], F32, name="sskl_sb")
    nc.sync.dma_start(sskl[:DR, :], d["sskl"][:, :])
    bm = []
    for u in range(NBLK):
        t = cpool.tile([P, 2 * P], BF16, name=f"bm{u}")
        nc.sync.dma_start(t[:], d["bmask"][u * P:(u + 1) * P, :])
        bm.append(t)

    # ---- residual stream ----
    hT = [hpool.tile([P, TLOC], F32, name=f"hT{i}") for i in range(KT)]

    for rep_i in range(getattr(c_, 'repeat', 1)):
        # ---- embedding gather + transpose to feature-major ----
        with tc.tile_pool(name="emb", bufs=1) as epool:
            for b in range(2):
                idx = epool.tile([P, 1], I32, name=f"idx{b}", tag="idx", bufs=2)
                nc.sync.dma_start(idx[:], d["tok"][b * P:(b + 1) * P, :])
                gth = epool.tile([P, DIM], F32, name=f"gth{b}", tag="gth", bufs=2)
                nc.gpsimd.indirect_dma_start(
                    out=gth[:], out_offset=None, in_=d["emb"][:, :],
                    in_offset=IndirectOffsetOnAxis(ap=idx[:, :1], axis=0))
                for k in range(KT):
                    tp = psum.tile([P, P], F32, name=f"tp{b}_{k}", tag="sc",
                                   bufs=2)
                    nc.tensor.transpose(tp[:], gth[:, k * P:(k + 1) * P], ident[:])
                    nc.vector.tensor_copy(hT[k][:, b * P:(b + 1) * P], tp[:])

        # ---- helpers ----
        def rms(tiles, out_tiles, pool, nm):
            """out = in * rsqrt(mean_over_partition_tiles(in^2) + 1e-6), bf16."""
            nkt = len(tiles)
            W = tiles[0].shape[-1]
            ssq_ps = psum.tile([1, W], F32, name=f"ssqp_{nm}", tag="attv", bufs=2)
            sqs = []
            for i, t in enumerate(tiles):
                sq = pool.tile([P, W], BF16, name=f"sq_{nm}_{i}", tag=f"sq_{nm}",
                               bufs=3)
                nc.vector.tensor_tensor(out=sq[:], in0=t[:], in1=t[:], op=ALU.mult)
                sqs.append(sq)
            for i, sq in enumerate(sqs):
                nc.tensor.matmul(ssq_ps[:1, :], ones_bf[:, :1], sq[:],
                                 start=(i == 0), stop=(i == nkt - 1))
            srt = spool.tile([1, W], F32, name=f"srt_{nm}", tag="srt", bufs=2)
            nc.scalar.activation(srt[:1, :], ssq_ps[:1, :], AF.Sqrt,
                                 bias=eps_t[:1, :1], scale=1.0 / (nkt * P))
            rcp = spool.tile([1, W], F32, name=f"rcp_{nm}", tag="rcp", bufs=2)
            nc.vector.reciprocal(rcp[:1, :], srt[:1, :])
            scb = pool.tile([P, W], F32, name=f"scb_{nm}", tag=f"scb_{nm}", bufs=1)
            nc.gpsimd.partition_broadcast(scb[:], rcp[:1, :])
            for i, t in enumerate(tiles):
                nc.vector.tensor_tensor(out=out_tiles[i][:], in0=t[:], in1=scb[:],
                                        op=ALU.mult)

        def rope(ps_ap, rows, cc, ss, outs, pool, nm):
            """ps_ap: [rows, W] fp32 (psum) with [e(32)|o(32)] row groups.
            outs: list of (bf16 out AP [64, W], row0 in ps)."""
            W = ps_ap.shape[-1]
            t1 = pool.tile([P, W], F32, name=f"rt1_{nm}", tag="rt1", bufs=2)
            t2 = pool.tile([P, W], F32, name=f"rt2_{nm}", tag="rt2", bufs=2)
            nc.vector.tensor_tensor(out=t1[:rows, :], in0=ps_ap[:rows, :],
                                    in1=cc[:rows, :], op=ALU.mult)
            for g in range(rows // 32):
                sg = g ^ 1
                nc.vector.tensor_tensor(
                    out=t2[g * 32:(g + 1) * 32, :],
                    in0=ps_ap[sg * 32:(sg + 1) * 32, :],
                    in1=ss[g * 32:(g + 1) * 32, :], op=ALU.mult)
            for out_ap, r0 in outs:
                nc.vector.tensor_tensor(out=out_ap, in0=t1[r0:r0 + 64, :],
                                        in1=t2[r0:r0 + 64, :], op=ALU.add)

        # =======================================================================
        for l in range(c_.L):
            # Pool scoping (LIFO): pA holds q/kpe through attention; pD holds
            # kv_norm through kvb; pB (xT) closes after q; pC (kv_a temps)
            # closes after kv norm; pE (v/attention/wo) closes after wo;
            # pF (FFN) is last.
            pA_cm = tc.tile_pool(name=f"qkv{l}", bufs=1)
            pA = pA_cm.__enter__()
            pD_cm = tc.tile_pool(name=f"kvnorm{l}", bufs=1)
            pD = pD_cm.__enter__()
            pB_cm = tc.tile_pool(name=f"x{l}", bufs=1)
            pB = pB_cm.__enter__()
            if True:
                # ---------- rms 1 -> xT ----------
                xTt = pB.tile([P, KT, TLOC], BF16, name=f"xT{l}", tag="xT")
                xT = [xTt[:, i, :] for i in range(KT)]
                rms(hT, xT, pB, f"a{l}")

                # ---------- local kv_a (own 256 tokens only) ----------
                wkva_sb = pB.tile([P, KT, KVRD], BF16, name=f"wkva{l}",
                                  tag="wkva", bufs=1)
                nc.sync.dma_start(
                    wkva_sb[:, :, :],
                    d[f"wkva{l}"].rearrange("(kt p) c -> p kt c", p=P))
                kvf_loc = pB.tile([P, KVT, TLOC], BF16, name=f"kvfl{l}",
                                  tag="kvfl")
                kpe_loc = pB.tile([64, TLOC], BF16, name=f"kpel{l}",
                                  tag="kpel")
                for m in range(KVT):
                    ps = psum.tile([P, TLOC], F32, name=f"kvps{l}_{m}",
                                   tag="mm", bufs=3)
                    for k in range(KT):
                        nc.tensor.matmul(
                            ps[:], wkva_sb[:, k, m * P:(m + 1) * P],
                            xT[k][:], start=(k == 0), stop=(k == KT - 1))
                    nc.vector.tensor_copy(kvf_loc[:, m, :], ps[:])
                ps = psum.tile([P, TLOC], F32, name=f"kpps{l}", tag="mm",
                               bufs=3)
                for k in range(KT):
                    nc.tensor.matmul(ps[:64, :], wkva_sb[:, k, KVR:KVRD],
                                     xT[k][:], start=(k == 0),
                                     stop=(k == KT - 1))
                rope(ps[:], 64, cckl, sskl, [(kpe_loc[:64, :], 0)], pB,
                     f"k{l}")

                # ---------- local kv rms norm ----------
                sqkv = pB.tile([P, KVT, TLOC], BF16, name=f"kvsq{l}",
                               tag="kvsq")
                for m in range(KVT):
                    nc.vector.tensor_tensor(out=sqkv[:, m, :],
                                            in0=kvf_loc[:, m, :],
                                            in1=kvf_loc[:, m, :], op=ALU.mult)
                ssqkv = psum.tile([1, TLOC], F32, name=f"kvssq{l}",
                                  tag="attv", bufs=2)
                for m in range(KVT):
                    nc.tensor.matmul(ssqkv[:1, :], ones_bf[:, :1],
                                     sqkv[:, m, :], start=(m == 0),
                                     stop=(m == KVT - 1))
                srtkv = spool.tile([1, TLOC], F32, name=f"kvsrt{l}",
                                   tag="srt", bufs=2)
                nc.scalar.activation(srtkv[:1, :], ssqkv[:1, :], AF.Sqrt,
                                     bias=eps_t[:1, :1], scale=1.0 / KVR)
                rcpkv = spool.tile([1, TLOC], F32, name=f"kvrcp{l}",
                                   tag="rcp", bufs=2)
                nc.vector.reciprocal(rcpkv[:1, :], srtkv[:1, :])
                scbkv = pB.tile([P, TLOC], F32, name=f"scbkv{l}",
                                tag="scbkv")
                nc.gpsimd.partition_broadcast(scbkv[:], rcpkv[:1, :])
                kvn_loc = pB.tile([P, KVT, TLOC], BF16, name=f"kvnl{l}",
                                  tag="kvnl")
                for m in range(KVT):
                    nc.vector.tensor_tensor(out=kvn_loc[:, m, :],
                                            in0=kvf_loc[:, m, :],
                                            in1=scbkv[:], op=ALU.mult)

                # ---------- AllGather (kvn_loc | kpe_loc) ----------
                ag_in = dram.tile([KVRD, TLOC], BF16, name=f"agin{l}",
                                  tag="agin", bufs=2)
                ag_out = dram.tile([NCOR * KVRD, TLOC], BF16, name=f"agout{l}",
                                   tag="agout", bufs=2,
                                   addr_space="Shared"
                                   if (NCOR > 4 and not getattr(c_, "mock_ag",
                                                                False))
                                   else "Local")
                nc.sync.dma_start(
                    ag_in[:KVR, :].rearrange("(m p) c -> p m c", p=P),
                    kvn_loc[:, :, :])
                nc.sync.dma_start(ag_in[KVR:KVRD, :], kpe_loc[:64, :])
                if getattr(c_, "mock_ag", False):
                    # timing-model variant: stand in for the AllGather with
                    # local DMA copies (data is wrong for r != my core)
                    for r in range(NCOR):
                        nc.sync.dma_start(ag_out[r * KVRD:(r + 1) * KVRD, :],
                                          ag_in[:, :])
                else:
                    nc.gpsimd.collective_compute(
                        "AllGather", ALU.bypass,
                        replica_groups=[list(range(NCOR))],
                        ins=[ag_in.opt()], outs=[ag_out.opt()])

                # ---------- q projection (overlaps the AllGather) ----------
                qn = [pA.tile([P, TLOC], BF16, name=f"qn{l}_{h}", tag=f"qn{h}")
                      for h in range(H)]
                qpe = [pA.tile([64, TLOC], BF16, name=f"qpe{l}_{h}", tag=f"qpe{h}")
                       for h in range(H)]
                MQ = H + c_.PE_T
                wq_r = d[f"wq{l}"].rearrange("(kt p) c -> p kt c", p=P)
                for mg in range((MQ + 3) // 4):
                    mw = min(512, MQ * P - mg * 512)
                    wqs = pB.tile([P, KT, 512], BF16, name=f"wq{l}_{mg}",
                                  tag="wq", bufs=2)
                    nc.sync.dma_start(wqs[:, :, :mw],
                                      wq_r[:, :, mg * 512:mg * 512 + mw])
                    for mi in range(4):
                        m = mg * 4 + mi
                        if m >= MQ:
                            break
                        ps = psum.tile([P, TLOC], F32, name=f"qps{l}_{m}",
                                       tag="mm", bufs=3)
                        for k in range(KT):
                            nc.tensor.matmul(ps[:],
                                             wqs[:, k, mi * P:(mi + 1) * P],
                                             xT[k][:], start=(k == 0),
                                             stop=(k == KT - 1))
                        if m < H:
                            nc.vector.tensor_copy(qn[m][:], ps[:])
                        else:
                            p = m - H
                            rope(ps[:], P, ccq, ssq,
                                 [(qpe[2 * p][:64, :], 0),
                                  (qpe[2 * p + 1][:64, :], 64)], pB, f"q{l}_{p}")

                # ---------- assemble gathered kv into SBUF ----------
                kpe = pA.tile([64, S], BF16, name=f"kpe{l}", tag="kpe")
                kvn_t = pD.tile([P, KVT, S], BF16, name=f"kvnt{l}",
                                tag="kvnt")
                for r in range(NCOR):
                    base = r * KVRD
                    nc.sync.dma_start(
                        kvn_t[:, :, r * TLOC:(r + 1) * TLOC],
                        ag_out[base:base + KVR, :].rearrange(
                            "(m p) c -> p m c", p=P))
                    nc.sync.dma_start(kpe[:64, r * TLOC:(r + 1) * TLOC],
                                      ag_out[base + KVR:base + KVRD, :])
            pB_cm.__exit__(*_nopex)

            pV_cm = tc.tile_pool(name=f"vres{l}", bufs=1)
            pV = pV_cm.__enter__()
            pE_cm = tc.tile_pool(name=f"vat{l}", bufs=1)
            pE = pE_cm.__enter__()
            if True:
                # ---------- wkv_b: k_nope (-> DRAM) and V (token-major) -------
                knd = dram.tile([H * P, S], BF16, name=f"knd{l}", tag="knd",
                                bufs=2)
                vsb = [pV.tile([P, H * DV], BF16, name=f"vsb{l}_{t}",
                               tag=f"vsb{t}") for t in range(NBLK)]
                bp = pE
                wkvb_r = d[f"wkvb{l}"].rearrange("(kt p) c -> p kt c", p=P)
                for mg in range(H // 4):
                    wn = bp.tile([P, KVT, 512], BF16, name=f"wn{l}_{mg}",
                                 tag="wvb", bufs=2)
                    nc.sync.dma_start(wn[:, :, :],
                                      wkvb_r[:, :, mg * 512:mg * 512 + 512])
                    for mi in range(4):
                        h = mg * 4 + mi
                        kst = bp.tile([P, S], BF16, name=f"kst{l}_{h}",
                                      tag="kst", bufs=2)
                        for cc in range(S // 512):
                            ps = psum.tile([P, 512], F32,
                                           name=f"knps{l}_{h}_{cc}",
                                           tag="mm", bufs=3)
                            for k in range(KVT):
                                nc.tensor.matmul(
                                    ps[:], wn[:, k, mi * P:(mi + 1) * P],
                                    kvn_t[:, k, cc * 512:cc * 512 + 512],
                                    start=(k == 0), stop=(k == KVT - 1))
                            nc.vector.tensor_copy(
                                kst[:, cc * 512:cc * 512 + 512], ps[:])
                        nc.sync.dma_start(knd[h * P:(h + 1) * P, :], kst[:])
                for vc in range(H * DV // 512):
                    wv = bp.tile([P, KVT, 512], BF16, name=f"wv{l}_{vc}",
                                 tag="wvb", bufs=2)
                    nc.sync.dma_start(
                        wv[:, :, :],
                        wkvb_r[:, :, H * c_.DN + vc * 512:
                               H * c_.DN + vc * 512 + 512])
                    for t in range(NBLK):
                        ps = psum.tile([P, 512], F32, name=f"vps{l}_{vc}_{t}",
                                       tag="mm", bufs=3)
                        for k in range(KVT):
                            nc.tensor.matmul(
                                ps[:], kvn_t[:, k, t * P:(t + 1) * P],
                                wv[:, k, :], start=(k == 0), stop=(k == KVT - 1))
                        nc.vector.tensor_copy(vsb[t][:, vc * 512:vc * 512 + 512],
                                              ps[:])

                # ---------- attention (both q-blocks in one 256-wide pass) ----
                aat = [pV.tile([P, TLOC], BF16, name=f"aat{l}_{i}", tag=f"aat{i}")
                       for i in range(KT)]
                ap = pE
                for h in range(H):
                    kn = ap.tile([P, NBLK, P], BF16, name=f"kn{l}_{h}",
                                 tag="kn", bufs=2)
                    nc.sync.dma_start(
                        kn[:, :, :],
                        knd[h * P:(h + 1) * P, :].rearrange(
                            "p (u c) -> p u c", c=P))
                    att_ps = psum.tile([P, TLOC], F32, name=f"atp{l}_{h}",
                                       tag="attv", bufs=2)
                    den_ps = psum.tile([1, TLOC], F32, name=f"dnp{l}_{h}",
                                       tag="den", bufs=1)
                    for kb in range(NBLK):
                        kcol = c_.col_of(kb)
                        kj = kcol // P
                        sc_ps = psum.tile([P, TLOC], F32,
                                          name=f"scp{l}_{h}_{kb}",
                                          tag="sc", bufs=2)
                        nc.tensor.matmul(sc_ps[:], kn[:, kj, :], qn[h][:],
                                         start=True, stop=False)
                        nc.tensor.matmul(sc_ps[:], kpe[:64, kcol:kcol + P],
                                         qpe[h][:64, :],
                                         start=False, stop=True)
                        e = ap.tile([P, TLOC], BF16, name=f"e{l}_{h}_{kb}",
                                    tag="e", bufs=6)
                        nc.scalar.activation(e[:], sc_ps[:], AF.Exp)
                        nc.vector.tensor_tensor(out=e[:], in0=e[:],
                                                in1=bm[kb][:], op=ALU.mult)
                        nc.tensor.matmul(att_ps[:],
                                         vsb[kj][:, h * DV:(h + 1) * DV],
                                         e[:], start=(kb == 0),
                                         stop=(kb == NBLK - 1))
                        nc.tensor.matmul(den_ps[:1, :], ones_bf[:, :1],
                                         e[:], start=(kb == 0),
                                         stop=(kb == NBLK - 1))
                    rcp = spool.tile([1, TLOC], F32, name=f"arc{l}_{h}",
                                     tag="rcp", bufs=2)
                    nc.vector.reciprocal(rcp[:1, :], den_ps[:1, :])
                    rb = ap.tile([P, TLOC], F32, name=f"rb{l}_{h}",
                                 tag="rb", bufs=2)
                    nc.gpsimd.partition_broadcast(rb[:], rcp[:1, :])
                    nc.vector.tensor_tensor(out=aat[h][:], in0=att_ps[:],
                                            in1=rb[:], op=ALU.mult)

                # ---------- wo + residual ----------
                op = pE
                if True:
                    wo_r = d[f"wo{l}"].rearrange("(kt p) c -> p kt c", p=P)
                    KH = min(8, KT)
                    for mg in range(KT // 4):
                        wos = op.tile([P, KT, 512], BF16, name=f"wo{l}_{mg}",
                                      tag="wo", bufs=1)
                        for kh in range(KT // KH):
                            nc.sync.dma_start(
                                wos[:, kh * KH:(kh + 1) * KH, :],
                                wo_r[:, kh * KH:(kh + 1) * KH,
                                     mg * 512:mg * 512 + 512])
                        for mi in range(4):
                            m = mg * 4 + mi
                            ps = psum.tile([P, TLOC], F32, name=f"ops{l}_{m}",
                                           tag="mm", bufs=3)
                            for k in range(KT):
                                nc.tensor.matmul(
                                    ps[:], wos[:, k, mi * P:(mi + 1) * P],
                                    aat[k][:], start=(k == 0), stop=(k == KT - 1))
                            nc.vector.tensor_tensor(out=hT[m][:], in0=hT[m][:],
                                                    in1=ps[:], op=ALU.add)

            pE_cm.__exit__(*_nopex)
            pV_cm.__exit__(*_nopex)
            pD_cm.__exit__(*_nopex)
            pA_cm.__exit__(*_nopex)

            pF_cm = tc.tile_pool(name=f"ffn{l}", bufs=1)
            pF = pF_cm.__enter__()
            if True:
                # ---------- FFN ----------
                fp = pF
                x2T = [pF.tile([P, TLOC], BF16, name=f"x2T{l}_{i}", tag=f"xT{i}_f")
                       for i in range(KT)]
                rms(hT, x2T, pF, f"f{l}")
                gat = [pF.tile([P, TLOC], BF16, name=f"gat{l}_{m}", tag=f"gat{m}")
                       for m in range(IT)]
                w1_r = d[f"w1{l}"].rearrange("(kt p) c -> p kt c", p=P)
                w3_r = d[f"w3{l}"].rearrange("(kt p) c -> p kt c", p=P)
                KH = min(8, KT)
                for mg in range(IT // 4):
                    w1s = fp.tile([P, KT, 512], BF16, name=f"w1{l}_{mg}",
                                  tag="w1", bufs=2)
                    w3s = fp.tile([P, KT, 512], BF16, name=f"w3{l}_{mg}",
                                  tag="w3", bufs=2)
                    for kh in range(KT // KH):
                        ksl = slice(kh * KH, (kh + 1) * KH)
                        nc.sync.dma_start(w1s[:, ksl, :],
                                          w1_r[:, ksl, mg * 512:mg * 512 + 512])
                        nc.sync.dma_start(w3s[:, ksl, :],
                                          w3_r[:, ksl, mg * 512:mg * 512 + 512])
                    for mi in range(4):
                        m = mg * 4 + mi
                        ups = psum.tile([P, TLOC], F32, name=f"ups{l}_{m}",
                                        tag="mm", bufs=3)
                        for k in range(KT):
                            nc.tensor.matmul(
                                ups[:], w1s[:, k, mi * P:(mi + 1) * P],
                                x2T[k][:], start=(k == 0), stop=(k == KT - 1))
                        sg = fp.tile([P, TLOC], BF16, name=f"sg{l}_{m}",
                                     tag="sg", bufs=4)
                        nc.scalar.activation(sg[:], ups[:], AF.Sigmoid)
                        su = fp.tile([P, TLOC], BF16, name=f"su{l}_{m}",
                                     tag="su", bufs=4)
                        nc.vector.tensor_tensor(out=su[:], in0=ups[:], in1=sg[:],
                                                op=ALU.mult)
                        gps = psum.tile([P, TLOC], F32, name=f"gps{l}_{m}",
                                        tag="mm", bufs=3)
                        for k in range(KT):
                            nc.tensor.matmul(
                                gps[:], w3s[:, k, mi * P:(mi + 1) * P],
                                x2T[k][:], start=(k == 0), stop=(k == KT - 1))
                        nc.vector.tensor_tensor(out=gat[m][:], in0=gps[:],
                                                in1=su[:], op=ALU.mult)
                # w2: k-outer accumulation, m-groups of 4 (psum tags borrowed
                # from the idle attention tags to stay within 8 banks)
                w2_r = d[f"w2{l}"].rearrange("(kt p) c -> p kt c", p=P)
                KG = max(1, IT // KH)
                for mg in range(KT // 4):
                    tags = [("mm", 3), ("mm", 3), ("sc", 2), ("attv", 2)]
                    pss = [psum.tile([P, TLOC], F32, name=f"yps{l}_{mg}_{mi}",
                                     tag=tags[mi][0], bufs=tags[mi][1])
                           for mi in range(4)]
                    for kg in range(KG):
                        w2t = fp.tile([P, KH, 512], BF16, name=f"w2{l}_{mg}_{kg}",
                                      tag="w2", bufs=3)
                        nc.sync.dma_start(
                            w2t[:, :, :],
                            w2_r[:, kg * KH:(kg + 1) * KH,
                                 mg * 512:mg * 512 + 512])
                        for ki in range(KH):
                            k = kg * KH + ki
                            for mi in range(4):
                                nc.tensor.matmul(
                                    pss[mi][:], w2t[:, ki, mi * P:(mi + 1) * P],
                                    gat[k][:], start=(k == 0),
                                    stop=(k == IT - 1))
                    for mi in range(4):
                        m = mg * 4 + mi
                        nc.vector.tensor_tensor(out=hT[m][:], in0=hT[m][:],
                                                in1=pss[mi][:], op=ALU.add)
            pF_cm.__exit__(*_nopex)

        # ---------- final norm + vocab head (token-major output) ----------
        with tc.tile_pool(name="head", bufs=1) as hp:
            xfT = [hp.tile([P, TLOC], BF16, name=f"xfT{i}", tag=f"xfT{i}")
                   for i in range(KT)]
            rms(hT, xfT, hp, "h")
            NV = VOCAB // 512 if VOCAB % 512 == 0 else VOCAB // 512 + 1
            hw_r = d["hw"].rearrange("(kt p) v -> p kt v", p=P)
            for vc in range(NV):
                vw = min(512, VOCAB - vc * 512)
                hws = hp.tile([P, KT, 512], BF16, name=f"hw_{vc}", tag="hw",
                              bufs=3)
                KH = min(8, KT)
                for kh in range(KT // KH):
                    ksl = slice(kh * KH, (kh + 1) * KH)
                    nc.sync.dma_start(hws[:, ksl, :vw],
                                      hw_r[:, ksl, vc * 512:vc * 512 + vw])
                for st in range(2):
                    ps = psum.tile([P, 512], F32, name=f"lps_{vc}_{st}", tag="mm",
                                   bufs=3)
                    for k in range(KT):
                        nc.tensor.matmul(ps[:, :vw],
                                         xfT[k][:, st * P:(st + 1) * P],
                                         hws[:, k, :vw],
                                         start=(k == 0), stop=(k == KT - 1))
                    lg = hp.tile([P, 512], F32, name=f"lg_{vc}_{st}", tag="lg",
                                 bufs=4)
                    nc.vector.tensor_copy(lg[:, :vw], ps[:, :vw])
                    nc.sync.dma_start(
                        d["out"][st * P:(st + 1) * P, vc * 512:vc * 512 + vw],
                        lg[:, :vw])

    stack.close()


# ---------------------------------------------------------------------------
# entry point
# ---------------------------------------------------------------------------

_CACHE = {}


def _get_nc(c_: Cfg):
    key = tuple(sorted(c_.__dict__.items()))
    if key not in _CACHE:
        nc = bacc.Bacc("TRN2", target_bir_lowering=False, debug=False,
                       num_devices=c_.n_cores)
        build(nc, c_)
        _CACHE[key] = nc
    return _CACHE[key]


def kernel(**inputs):
    c_ = Cfg()
    nc = _get_nc(c_)
    in_maps = host_prepare(inputs, c_)
    res = run_bass_kernel_spmd(nc, in_maps,
                               core_ids=list(range(c_.n_cores)))
    out = np.zeros((1, c_.S, c_.VOCAB), np.float32)
    for c in range(c_.n_cores):
        out[0, c_.pos_of(c), :] = res.results[c]["logits"]
    return out

